# revision 1
# baseline (speedup 1.0000x reference)
"""Trainium2 Bass kernel for DependencyGNN (2-layer GCN + global mean pool).

Distribution: nodes sharded contiguously across 8 cores (25000 each, padded to
25088 = 196*128). Edges assigned to the owner of their dst node, grouped into
128-node dst windows, padded per-window to a multiple of 128 (window tile
counts equalized across cores so all cores run one SPMD program).

Per core:
  stage 1: h1_lin = x_shard @ W1            (PE, xT supplied feature-major)
  AllGather h1_lin -> h1full (replicated)
  layer 1: gather h1full[src] per edge tile; aggregate via PE matmul with a
           one-hot*norm selection matrix; + self-loop + b1; relu -> h1
  AllGather h1 -> h1rfull
  layer 2: same aggregation on h1rfull -> agg2pre (pre-W2, since W2/b2/pooling
           commute past the linear aggregation)
Host epilogue: segment-mean-pool agg2pre over graphs, @W2 + b2.
"""
import numpy as np

N_NODES = 200000
N_EDGES = 400000
IN_CH, HID_CH, OUT_CH = 768, 256, 256
NUM_GRAPHS = 8000
N_CORES = 8
P = 128
NPC = N_NODES // N_CORES            # 25000 nodes per core
NT = (NPC + P - 1) // P             # 196 m-tiles per core
NPC_PAD = NT * P                    # 25088
K_TILES = IN_CH // P                # 6


def _prep(x, W1, b1, W2, b2, edge_index, batch):
    x = np.asarray(x, dtype=np.float32)
    W1 = np.asarray(W1, dtype=np.float32)
    b1 = np.asarray(b1, dtype=np.float32)
    W2 = np.asarray(W2, dtype=np.float32)
    b2 = np.asarray(b2, dtype=np.float32)
    src = np.asarray(edge_index[0], dtype=np.int64)
    dst = np.asarray(edge_index[1], dtype=np.int64)
    batch = np.asarray(batch, dtype=np.int64)

    # degree (dst-side, incl self-loop), symmetric norm
    deg = np.bincount(dst, minlength=N_NODES).astype(np.float64) + 1.0
    dinv = (1.0 / np.sqrt(deg)).astype(np.float32)
    enorm = dinv[src] * dinv[dst]
    dinv2 = (dinv * dinv).astype(np.float32)

    # global row index in the AG-concatenated (padded) table
    def grow(n):
        return (n // NPC) * NPC_PAD + (n % NPC)

    owner = dst // NPC
    win = (dst % NPC) // P
    # per (core, window) edge counts -> equalized tile counts
    cnt = np.zeros((N_CORES, NT), dtype=np.int64)
    np.add.at(cnt, (owner, win), 1)
    tw = np.maximum((cnt.max(axis=0) + P - 1) // P, 1).astype(np.int64)  # [NT]
    ET = int(tw.sum())
    col_base = np.zeros(NT, dtype=np.int64)
    col_base[1:] = np.cumsum(tw)[:-1]
    win_of_col = np.repeat(np.arange(NT), tw)            # [ET]

    gsrc = np.zeros((N_CORES, P, ET), dtype=np.int32)
    gdst = np.full((N_CORES, P, ET), -1.0, dtype=np.float32)
    gnrm = np.zeros((N_CORES, P, ET), dtype=np.float32)

    order = np.lexsort((dst, win, owner))
    so, sw = owner[order], win[order]
    ssrc, sdst, snorm = src[order], dst[order], enorm[order]
    # position within (core, window) group
    grp = so * NT + sw
    first = np.zeros(N_CORES * NT, dtype=np.int64)
    np.add.at(first, grp, 1)
    starts = np.zeros(N_CORES * NT, dtype=np.int64)
    starts[1:] = np.cumsum(first)[:-1]
    pos = np.arange(len(order)) - starts[grp]
    col = col_base[sw] + pos // P
    row = pos % P
    gsrc[so, row, col] = grow(ssrc).astype(np.int32)
    gdst[so, row, col] = (sdst % NPC - sw * P).astype(np.float32)
    gnrm[so, row, col] = snorm

    # per-core tensors
    in_maps = []
    xpad = np.zeros((NPC_PAD, IN_CH), dtype=np.float32)
    d2 = np.zeros((P, NT), dtype=np.float32)
    for c in range(N_CORES):
        lo = c * NPC
        xpad[:NPC] = x[lo:lo + NPC]
        xt = np.ascontiguousarray(xpad.T)                 # [768, 25088]
        d2[:] = 0.0
        dv = dinv2[lo:lo + NPC]
        d2c = np.zeros(NPC_PAD, dtype=np.float32)
        d2c[:NPC] = dv
        d2 = np.ascontiguousarray(d2c.reshape(NT, P).T)   # [128, NT]
        in_maps.append({
            "xt": xt.copy(),
            "w1": W1,
            "gsrc": np.ascontiguousarray(gsrc[c]),
            "gdst": np.ascontiguousarray(gdst[c]),
            "gnrm": np.ascontiguousarray(gnrm[c]),
            "dinv2": d2.copy(),
            "b1b": np.tile(b1[None, :], (P, 1)),
            "iota": np.tile(np.arange(P, dtype=np.float32)[None, :], (P, 1)),
        })

    meta = {"ET": ET, "tw": tw.tolist()}
    counts = np.bincount(batch, minlength=NUM_GRAPHS).astype(np.int64)
    seg_starts = np.minimum(
        np.searchsorted(batch, np.arange(NUM_GRAPHS)), N_NODES - 1
    )
    aux = {"W2": W2, "b2": b2, "counts": counts, "seg_starts": seg_starts}
    return meta, in_maps, aux


def _build(meta):
    import concourse.bass as bass
    import concourse.bacc as bacc
    import concourse.mybir as mybir
    import concourse.tile as tile

    ET = meta["ET"]
    tw = meta["tw"]
    f32 = mybir.dt.float32

    nc = bacc.Bacc()
    xt = nc.declare_dram_parameter("xt", [IN_CH, NPC_PAD], f32, isOutput=False)
    w1 = nc.declare_dram_parameter("w1", [IN_CH, HID_CH], f32, isOutput=False)
    gsrc = nc.declare_dram_parameter("gsrc", [P, ET], mybir.dt.int32, isOutput=False)
    gdst = nc.declare_dram_parameter("gdst", [P, ET], f32, isOutput=False)
    gnrm = nc.declare_dram_parameter("gnrm", [P, ET], f32, isOutput=False)
    dinv2 = nc.declare_dram_parameter("dinv2", [P, NT], f32, isOutput=False)
    b1b = nc.declare_dram_parameter("b1b", [P, HID_CH], f32, isOutput=False)
    iota = nc.declare_dram_parameter("iota", [P, P], f32, isOutput=False)
    out_pre = nc.declare_dram_parameter("out_pre", [NPC_PAD, HID_CH], f32, isOutput=True)

    h1l = nc.dram_tensor("h1l", [NPC_PAD, HID_CH], f32)                       # ag1 in
    h1full = nc.dram_tensor("h1full", [NPC_PAD * N_CORES, HID_CH], f32, addr_space="Shared")
    h1own = nc.dram_tensor("h1own", [NPC_PAD, HID_CH], f32)                   # ag2 in
    h1rfull = nc.dram_tensor("h1rfull", [NPC_PAD * N_CORES, HID_CH], f32, addr_space="Shared")

    # ---- stage 1: h1l = x @ W1 ----
    with tile.TileContext(nc) as tc:
        with (
            tc.tile_pool(name="s1", bufs=3) as sbuf,
            tc.tile_pool(name="s1c", bufs=1) as cbuf,
            tc.tile_pool(name="p1", bufs=2, space="PSUM") as psum,
        ):
            w1_t = cbuf.tile([P, K_TILES, HID_CH], f32)
            nc.sync.dma_start(out=w1_t[:], in_=w1[:].rearrange("(a k) n -> k a n", k=P))
            for m in range(NT):
                xt_t = sbuf.tile([P, K_TILES, P], f32, tag="xt")
                nc.sync.dma_start(
                    out=xt_t[:],
                    in_=xt[:, m * P:(m + 1) * P].rearrange("(a k) m -> k a m", k=P),
                )
                acc = psum.tile([P, HID_CH], f32, tag="acc")
                for k in range(K_TILES):
                    nc.tensor.matmul(
                        acc[:], lhsT=xt_t[:, k, :], rhs=w1_t[:, k, :],
                        start=(k == 0), stop=(k == K_TILES - 1),
                    )
                h = sbuf.tile([P, HID_CH], f32, tag="h")
                nc.vector.tensor_copy(out=h[:], in_=acc[:])
                nc.sync.dma_start(out=h1l[m * P:(m + 1) * P, :], in_=h[:])

    cc_sem = nc.semaphore("cc_sem").__enter__()
    nc.gpsimd.collective_compute(
        "AllGather", mybir.AluOpType.bypass,
        ins=[h1l[:]], outs=[h1full[:]],
        replica_groups=[list(range(N_CORES))],
    ).then_inc(cc_sem, 1)
    nc.gpsimd.wait_ge(cc_sem, 1)

    # ---- aggregation layer (shared builder) ----
    def agg_layer(table, local_in, dest, relu_bias):
        with tile.TileContext(nc) as tc:
            with (
                tc.tile_pool(name="sa", bufs=4) as sbuf,
                tc.tile_pool(name="sac", bufs=1) as cbuf,
                tc.tile_pool(name="pa", bufs=2, space="PSUM") as psum,
            ):
                gsrc_t = cbuf.tile([P, ET], mybir.dt.int32)
                gdst_t = cbuf.tile([P, ET], f32)
                gnrm_t = cbuf.tile([P, ET], f32)
                iota_t = cbuf.tile([P, P], f32)
                d2_t = cbuf.tile([P, NT], f32)
                b1_t = cbuf.tile([P, HID_CH], f32)
                nc.sync.dma_start(out=gsrc_t[:], in_=gsrc[:])
                nc.sync.dma_start(out=gdst_t[:], in_=gdst[:])
                nc.sync.dma_start(out=gnrm_t[:], in_=gnrm[:])
                nc.sync.dma_start(out=iota_t[:], in_=iota[:])
                nc.sync.dma_start(out=d2_t[:], in_=dinv2[:])
                nc.sync.dma_start(out=b1_t[:], in_=b1b[:])
                col = 0
                for w in range(NT):
                    acc = psum.tile([P, HID_CH], f32, tag="acc")
                    for t in range(tw[w]):
                        msg = sbuf.tile([P, HID_CH], f32, tag="msg")
                        nc.gpsimd.indirect_dma_start(
                            out=msg[:], out_offset=None, in_=table[:],
                            in_offset=bass.IndirectOffsetOnAxis(
                                ap=gsrc_t[:, col:col + 1], axis=0),
                        )
                        pt = sbuf.tile([P, P], f32, tag="pt")
                        nc.vector.tensor_tensor(
                            out=pt[:], in0=gdst_t[:, col:col + 1].to_broadcast([P, P]),
                            in1=iota_t[:], op=mybir.AluOpType.is_equal,
                        )
                        nc.vector.tensor_tensor(
                            out=pt[:], in0=pt[:],
                            in1=gnrm_t[:, col:col + 1].to_broadcast([P, P]),
                            op=mybir.AluOpType.mult,
                        )
                        nc.tensor.matmul(
                            acc[:], lhsT=pt[:], rhs=msg[:],
                            start=(t == 0), stop=(t == tw[w] - 1),
                        )
                        col += 1
                    # epilogue: + dinv2 * local_in  (+ b1, relu for layer 1)
                    loc = sbuf.tile([P, HID_CH], f32, tag="loc")
                    nc.sync.dma_start(out=loc[:], in_=local_in[w * P:(w + 1) * P, :])
                    tmp = sbuf.tile([P, HID_CH], f32, tag="tmp")
                    nc.vector.tensor_tensor(
                        out=tmp[:], in0=loc[:],
                        in1=d2_t[:, w:w + 1].to_broadcast([P, HID_CH]),
                        op=mybir.AluOpType.mult,
                    )
                    nc.vector.tensor_tensor(
                        out=tmp[:], in0=tmp[:], in1=acc[:], op=mybir.AluOpType.add,
                    )
                    outt = sbuf.tile([P, HID_CH], f32, tag="outt")
                    if relu_bias:
                        nc.vector.tensor_tensor(
                            out=tmp[:], in0=tmp[:], in1=b1_t[:], op=mybir.AluOpType.add,
                        )
                        nc.scalar.activation(
                            out=outt[:], in_=tmp[:],
                            func=mybir.ActivationFunctionType.Relu,
                        )
                    else:
                        nc.vector.tensor_copy(out=outt[:], in_=tmp[:])
                    nc.sync.dma_start(out=dest[w * P:(w + 1) * P, :], in_=outt[:])

    agg_layer(h1full, h1l, h1own, relu_bias=True)

    nc.gpsimd.collective_compute(
        "AllGather", mybir.AluOpType.bypass,
        ins=[h1own[:]], outs=[h1rfull[:]],
        replica_groups=[list(range(N_CORES))],
    ).then_inc(cc_sem, 1)
    nc.gpsimd.wait_ge(cc_sem, 2)

    agg_layer(h1rfull, h1own, out_pre, relu_bias=False)

    nc.finalize()
    return nc


def kernel(**inputs):
    from concourse.bass_utils import run_bass_kernel_spmd

    meta, in_maps, aux = _prep(
        inputs["x"], inputs["W1"], inputs["b1"], inputs["W2"], inputs["b2"],
        inputs["edge_index"], inputs["batch"],
    )
    nc = _build(meta)
    res = run_bass_kernel_spmd(nc, in_maps, list(range(N_CORES)))
    pre = np.concatenate(
        [res.results[c]["out_pre"][:NPC] for c in range(N_CORES)], axis=0
    )  # [N_NODES, 256] aggregated pre-W2 layer-2 features
    # host epilogue: mean pool per graph, then @W2 + b2
    counts = aux["counts"]
    sums = np.add.reduceat(pre, aux["seg_starts"], axis=0)
    sums[counts == 0] = 0.0
    pooled = sums / np.maximum(counts, 1)[:, None]
    out = pooled.astype(np.float32) @ aux["W2"] + aux["b2"]
    out[counts == 0] = 0.0
    return out.astype(np.float32)



# revision 12
# speedup vs baseline: 1.4271x; 1.4271x over previous
"""Trainium2 Bass kernel for DependencyGNN (2-layer GCN + global mean pool).

Distribution: nodes sharded contiguously across 8 cores (25000 each). Within a
core, nodes are re-ranked by ascending in-degree and chopped into 196 windows
of 128 output slots; a window's edge tiles (128 edges each, self-loops included
as edges) aggregate via one-hot matmuls into PSUM. The GCN symmetric norm
dinv[src]*dinv[dst] is separable, so each layer's node table stores
dinv-prescaled rows and the epilogue applies the dst-side dinv — no per-edge
scaling anywhere.

Cross-core src rows move via a compact AllToAll halo exchange (~23MB payload
instead of a 205MB full AllGather): each core gathers the unique rows every
peer needs from its bf16 table into a send buffer, AllToAll delivers them into
the halo region of the receiver's table, and edge gathers index own+halo rows
uniformly.

Everything on device is bf16 except PSUM accumulation and the final out_pre
(f32). Host epilogue: un-permute rows, segment-mean-pool over graphs, @W2+b2.
"""
import numpy as np
import ml_dtypes

N_NODES = 200000
N_EDGES = 400000
IN_CH, HID_CH, OUT_CH = 768, 256, 256
NUM_GRAPHS = 8000
N_CORES = 8
P = 128
NPC = N_NODES // N_CORES            # 25000 nodes per core
NT = (NPC + P - 1) // P             # 196 windows per core
NPC_PAD = NT * P                    # 25088
K_TILES = IN_CH // P                # 6

BF16 = ml_dtypes.bfloat16
F8 = ml_dtypes.float8_e4m3


def _prep(x, W1, b1, W2, b2, edge_index, batch):
    x = np.asarray(x, dtype=np.float32)
    W1 = np.asarray(W1, dtype=np.float32)
    b1 = np.asarray(b1, dtype=np.float32)
    W2 = np.asarray(W2, dtype=np.float32)
    b2 = np.asarray(b2, dtype=np.float32)
    src = np.asarray(edge_index[0], dtype=np.int64)
    dst = np.asarray(edge_index[1], dtype=np.int64)
    batch = np.asarray(batch, dtype=np.int64)

    # symmetric norm: deg on dst incl self-loop
    deg = np.bincount(dst, minlength=N_NODES).astype(np.float64) + 1.0
    dinv = (1.0 / np.sqrt(deg)).astype(np.float32)

    e_so = src // NPC                   # src owner
    e_do = dst // NPC                   # dst owner

    # ---- per-core rank permutation: sort local nodes by (in-deg+1) asc ----
    # rank[c][local] = slot order; windows = consecutive runs of 128 ranks.
    # Ascending-degree contiguous packing makes per-window edge-slot loads
    # near-multiples of 128 (minimal tile padding).
    sizes = (deg - 1.0).astype(np.int64)          # in-degree (self via loc)
    rank = np.empty(N_NODES, dtype=np.int64)      # rank within own core
    loads = np.zeros((N_CORES, NT), dtype=np.int64)
    for c in range(N_CORES):
        lo = c * NPC
        s = sizes[lo:lo + NPC]
        order = np.argsort(s, kind="stable")      # local ids by asc size
        r = np.empty(NPC, dtype=np.int64)
        r[order] = np.arange(NPC)
        rank[lo:lo + NPC] = r
        # per-window load = sum of sizes of its nodes
        w_of = r // P
        np.add.at(loads[c], w_of, s)
    tw = (loads.max(axis=0) + P - 1) // P          # shared [NT]; 0 allowed
    ET = int(tw.sum())
    col_base = np.zeros(NT, dtype=np.int64)
    col_base[1:] = np.cumsum(tw)[:-1]

    # ---- halo exchange lists ----
    # U[s][r]: sorted unique local src ids (s-owned) needed by receiver r.
    rem = e_so != e_do
    rs, rr = e_so[rem], e_do[rem]
    rrk = rank[src[rem]]                # rank within sender core
    # group by (s, r): unique (s, r, rank) -> columns ascend by rank
    key = (rs * N_CORES + rr) * NPC + rrk
    ukey = np.unique(key)
    u_sr = ukey // NPC
    u_rank = ukey % NPC
    u_s = u_sr // N_CORES
    u_r = u_sr % N_CORES
    grp_cnt = np.bincount(u_sr, minlength=N_CORES * N_CORES)
    CHUNK = int(-(-grp_cnt.max() // P) * P)
    # position of each unique row within its (s,r) group
    grp_start = np.zeros(N_CORES * N_CORES, dtype=np.int64)
    grp_start[1:] = np.cumsum(grp_cnt)[:-1]
    u_pos = np.arange(len(ukey)) - grp_start[u_sr]
    TROWS = NPC_PAD + N_CORES * CHUNK
    SCOLS = N_CORES * CHUNK // P

    # sidx[s]: [128, SCOLS] rows (in s's rank order) to gather into sendbuf
    sidx = np.zeros((N_CORES, N_CORES * CHUNK), dtype=np.int32)
    sidx[u_s, u_r * CHUNK + u_pos] = u_rank.astype(np.int32)
    # columns whose rows are all written after stage-1 window 97 (half point)
    half_ok = (sidx.reshape(N_CORES, SCOLS, P).max(axis=2) // P <= 97).all(axis=0)

    # map each remote edge's src -> table row on the receiver
    # row = NPC_PAD + s*CHUNK + pos(u in U[s][r])
    epos = np.searchsorted(ukey, key)             # index into ukey per edge
    erow = np.empty(N_EDGES, dtype=np.int64)
    erow[rem] = NPC_PAD + rs * CHUNK + u_pos[epos]
    loc_mask = ~rem
    erow[loc_mask] = rank[src[loc_mask]]

    # ---- edge tiles (real edges only), grouped by dst window ----
    # records: (core, window, slot, table_row)
    all_dst = dst
    all_row = erow
    a_c = all_dst // NPC
    a_rk = rank[all_dst]
    a_w = a_rk // P
    a_slot = a_rk % P

    gsrc = np.zeros((N_CORES, P, ET), dtype=np.int32)
    gdst = np.full((N_CORES, P, ET), -1.0, dtype=BF16)

    order = np.lexsort((a_rk, a_w, a_c))
    so_c, so_w = a_c[order], a_w[order]
    so_row, so_slot = all_row[order], a_slot[order]
    grp = so_c * NT + so_w
    cnt = np.bincount(grp, minlength=N_CORES * NT)
    starts = np.zeros(N_CORES * NT, dtype=np.int64)
    starts[1:] = np.cumsum(cnt)[:-1]
    pos = np.arange(len(order)) - starts[grp]
    tile_col = col_base[so_w] + pos // P
    tile_row = pos % P
    gsrc[so_c, tile_row, tile_col] = so_row.astype(np.int32)
    gdst[so_c, tile_row, tile_col] = so_slot.astype(BF16)

    # ---- per-core tensors ----
    in_maps = []
    iota = np.tile(np.arange(P, dtype=BF16)[None, :], (P, 1))
    b1b = np.tile(b1[None, :].astype(BF16), (P, 1))
    w1bf = W1.astype(BF16)
    ranks_out = []
    for c in range(N_CORES):
        lo = c * NPC
        r = rank[lo:lo + NPC]
        ranks_out.append(r)
        # x rows in rank order, padded; tiled [m*128+p, k*128+j] layout
        xp = np.zeros((NPC_PAD, IN_CH), dtype=np.float32)
        xp[r] = x[lo:lo + NPC]
        xt = np.ascontiguousarray(
            xp.reshape(NT, P, K_TILES, P).transpose(0, 3, 2, 1)
            .reshape(NT * P, IN_CH)
        ).astype(BF16)
        dcol = np.zeros(NPC_PAD, dtype=np.float32)
        dcol[r] = dinv[lo:lo + NPC]
        dcol = np.ascontiguousarray(dcol.reshape(NT, P).T)     # [128, NT]
        in_maps.append({
            "xt": xt,
            "w1": w1bf,
            "gsrc": np.ascontiguousarray(gsrc[c]),
            "gdst": np.ascontiguousarray(gdst[c]),
            "sidx": np.ascontiguousarray(
                sidx[c].reshape(SCOLS, P).T),                   # [128, SCOLS]
            "dinvcol": dcol,
            "dinv2col": np.ascontiguousarray(dcol * dcol),
            "b1b": b1b,
            "iota": iota,
            "eye": np.eye(P, dtype=F8),
        })

    meta = {"ET": ET, "tw": tw.tolist(), "CHUNK": CHUNK,
            "TROWS": TROWS, "SCOLS": SCOLS,
            "b1_zero": bool(not np.any(b1)),
            "half_cols": np.flatnonzero(half_ok).tolist()}
    counts = np.bincount(batch, minlength=NUM_GRAPHS).astype(np.int64)
    seg_starts = np.minimum(
        np.searchsorted(batch, np.arange(NUM_GRAPHS)), N_NODES - 1
    )
    aux = {"W2": W2, "b2": b2, "counts": counts, "seg_starts": seg_starts,
           "ranks": ranks_out}
    return meta, in_maps, aux


def _build(meta):
    import concourse.bass as bass
    import concourse.bacc as bacc
    import concourse.mybir as mybir
    import concourse.tile as tile

    ET = meta["ET"]
    tw = meta["tw"]
    CHUNK = meta["CHUNK"]
    TROWS = meta["TROWS"]
    SCOLS = meta["SCOLS"]
    b1_zero = meta["b1_zero"]
    half_cols = set(meta["half_cols"])
    f32 = mybir.dt.float32
    bf16 = mybir.dt.bfloat16
    f8 = mybir.dt.float8e4
    i32 = mybir.dt.int32

    nc = bacc.Bacc()
    xt = nc.declare_dram_parameter("xt", [NT * P, IN_CH], bf16, isOutput=False)
    w1 = nc.declare_dram_parameter("w1", [IN_CH, HID_CH], bf16, isOutput=False)
    gsrc = nc.declare_dram_parameter("gsrc", [P, ET], i32, isOutput=False)
    gdst = nc.declare_dram_parameter("gdst", [P, ET], bf16, isOutput=False)
    sidx = nc.declare_dram_parameter("sidx", [P, SCOLS], i32, isOutput=False)
    dinvcol = nc.declare_dram_parameter("dinvcol", [P, NT], f32, isOutput=False)
    dinv2col = nc.declare_dram_parameter("dinv2col", [P, NT], f32, isOutput=False)
    b1b = nc.declare_dram_parameter("b1b", [P, HID_CH], bf16, isOutput=False)
    iota = nc.declare_dram_parameter("iota", [P, P], bf16, isOutput=False)
    eye = nc.declare_dram_parameter("eye", [P, P], f8, isOutput=False)
    out_pre = nc.declare_dram_parameter("out_pre", [NPC_PAD, HID_CH], f32, isOutput=True)

    table1 = nc.dram_tensor("table1", [TROWS, HID_CH], f8)
    table2 = nc.dram_tensor("table2", [TROWS, HID_CH], f8)
    send1 = nc.dram_tensor("send1", [N_CORES * CHUNK, HID_CH], f8)
    send2 = nc.dram_tensor("send2", [N_CORES * CHUNK, HID_CH], f8)

    relu = mybir.ActivationFunctionType.Relu
    fcopy = mybir.ActivationFunctionType.Copy

    # ---- stage 1: table1[own] = dinv * (x @ W1) ----
    with tile.TileContext(nc) as tc:
        with (
            tc.tile_pool(name="s1", bufs=3) as sbuf,
            tc.tile_pool(name="sg1", bufs=4) as gbuf,
            tc.tile_pool(name="s1c", bufs=1) as cbuf,
            tc.tile_pool(name="p1", bufs=2, space="PSUM") as psum,
        ):
            w1_t = cbuf.tile([P, K_TILES, HID_CH], bf16)
            nc.sync.dma_start(
                out=w1_t[:], in_=w1[:].rearrange("(k a) n -> a k n", k=K_TILES)
            )
            dcol1 = cbuf.tile([P, NT], f32)
            nc.sync.dma_start(out=dcol1[:], in_=dinvcol[:])
            si1 = cbuf.tile([P, SCOLS], i32)
            nc.sync.dma_start(out=si1[:], in_=sidx[:])

            def emit_send1(cols):
                for t in cols:
                    g1 = gbuf.tile([P, HID_CH], f8, tag="g1")
                    nc.gpsimd.indirect_dma_start(
                        out=g1[:], out_offset=None, in_=table1[:],
                        in_offset=bass.IndirectOffsetOnAxis(
                            ap=si1[:, t:t + 1], axis=0),
                    )
                    nc.sync.dma_start(
                        out=send1[t * P:(t + 1) * P, :], in_=g1[:]
                    )

            for m in range(NT):
                xt_t = sbuf.tile([P, IN_CH], bf16, tag="xt")
                nc.sync.dma_start(out=xt_t[:], in_=xt[m * P:(m + 1) * P, :])
                acc = psum.tile([P, HID_CH], f32, tag="acc")
                for k in range(K_TILES):
                    nc.tensor.matmul(
                        acc[:], lhsT=xt_t[:, k * P:(k + 1) * P],
                        rhs=w1_t[:, k, :],
                        start=(k == 0), stop=(k == K_TILES - 1),
                    )
                h = sbuf.tile([P, HID_CH], f8, tag="h")
                nc.scalar.activation(
                    out=h[:], in_=acc[:], func=fcopy, scale=dcol1[:, m:m + 1]
                )
                nc.sync.dma_start(out=table1[m * P:(m + 1) * P, :], in_=h[:])
                if m == 97:
                    emit_send1([t for t in range(SCOLS) if t in half_cols])
            emit_send1([t for t in range(SCOLS) if t not in half_cols])

    cc_sem = nc.semaphore("cc_sem").__enter__()

    # ---- send gather: sendbuf = table[sidx] (own rows only) ----
    def send_gather(table, send):
        with tile.TileContext(nc) as tc:
            with (
                tc.tile_pool(name="sg", bufs=4) as sbuf,
                tc.tile_pool(name="sgc", bufs=1) as cbuf,
            ):
                si = cbuf.tile([P, SCOLS], i32)
                nc.sync.dma_start(out=si[:], in_=sidx[:])
                for t in range(SCOLS):
                    g = sbuf.tile([P, HID_CH], f8, tag="g")
                    nc.gpsimd.indirect_dma_start(
                        out=g[:], out_offset=None, in_=table[:],
                        in_offset=bass.IndirectOffsetOnAxis(
                            ap=si[:, t:t + 1], axis=0),
                    )
                    nc.sync.dma_start(
                        out=send[t * P:(t + 1) * P, :], in_=g[:]
                    )

    # ---- aggregation layer ----
    def agg_layer(table_in, dest, layer1):
        with tile.TileContext(nc) as tc:
            with (
                tc.tile_pool(name="sa", bufs=4) as sbuf,
                tc.tile_pool(name="sal", bufs=4) as lbuf,
                tc.tile_pool(name="sac", bufs=1) as cbuf,
                tc.tile_pool(name="pa", bufs=2, space="PSUM") as psum,
            ):
                gsrc_t = cbuf.tile([P, ET], i32)
                gdst_t = cbuf.tile([P, ET], bf16)
                iota_t = cbuf.tile([P, P], bf16)
                eye_t = cbuf.tile([P, P], f8)
                dcol = cbuf.tile([P, NT], f32)
                d2col = cbuf.tile([P, NT], f32)
                b1_t = cbuf.tile([P, HID_CH], bf16)
                nc.sync.dma_start(out=gsrc_t[:], in_=gsrc[:])
                nc.sync.dma_start(out=gdst_t[:], in_=gdst[:])
                nc.sync.dma_start(out=iota_t[:], in_=iota[:])
                nc.sync.dma_start(out=eye_t[:], in_=eye[:])
                nc.sync.dma_start(out=dcol[:], in_=dinvcol[:])
                nc.sync.dma_start(out=d2col[:], in_=dinv2col[:])
                nc.sync.dma_start(out=b1_t[:], in_=b1b[:])
                col = 0
                for w in range(NT):
                    acc = psum.tile([P, HID_CH], f32, tag="acc")
                    # self-loop term: acc = I @ table_in[window rows]
                    loc = lbuf.tile([P, HID_CH], f8, tag="loc")
                    nc.sync.dma_start(
                        out=loc[:], in_=table_in[w * P:(w + 1) * P, :])
                    nc.tensor.matmul(
                        acc[:], lhsT=eye_t[:], rhs=loc[:],
                        start=True, stop=(tw[w] == 0),
                    )
                    for t in range(tw[w]):
                        msg = sbuf.tile([P, HID_CH], f8, tag="msg")
                        nc.gpsimd.indirect_dma_start(
                            out=msg[:], out_offset=None, in_=table_in[:],
                            in_offset=bass.IndirectOffsetOnAxis(
                                ap=gsrc_t[:, col:col + 1], axis=0),
                        )
                        pt = sbuf.tile([P, P], f8, tag="pt")
                        nc.vector.tensor_tensor(
                            out=pt[:],
                            in0=gdst_t[:, col:col + 1].to_broadcast([P, P]),
                            in1=iota_t[:], op=mybir.AluOpType.is_equal,
                        )
                        nc.tensor.matmul(
                            acc[:], lhsT=pt[:], rhs=msg[:],
                            start=False, stop=(t == tw[w] - 1),
                        )
                        col += 1
                    if layer1 and b1_zero:
                        # table2 = relu(dinv^2 * acc)  (relu commutes with
                        # the positive per-node scale)
                        t2 = sbuf.tile([P, HID_CH], f8, tag="t2")
                        nc.scalar.activation(
                            out=t2[:], in_=acc[:], func=relu,
                            scale=d2col[:, w:w + 1],
                        )
                        nc.sync.dma_start(
                            out=dest[w * P:(w + 1) * P, :], in_=t2[:]
                        )
                    elif layer1:
                        # table2 = Relu((Copy(acc*dinv) + b1) * dinv)
                        v = sbuf.tile([P, HID_CH], bf16, tag="v")
                        nc.scalar.activation(
                            out=v[:], in_=acc[:], func=fcopy,
                            scale=dcol[:, w:w + 1],
                        )
                        wv = sbuf.tile([P, HID_CH], bf16, tag="wv")
                        nc.vector.tensor_tensor(
                            out=wv[:], in0=v[:], in1=b1_t[:],
                            op=mybir.AluOpType.add,
                        )
                        t2 = sbuf.tile([P, HID_CH], f8, tag="t2")
                        nc.scalar.activation(
                            out=t2[:], in_=wv[:], func=relu,
                            scale=dcol[:, w:w + 1],
                        )
                        nc.sync.dma_start(
                            out=dest[w * P:(w + 1) * P, :], in_=t2[:]
                        )
                    else:
                        o = sbuf.tile([P, HID_CH], f32, tag="o")
                        nc.scalar.activation(
                            out=o[:], in_=acc[:], func=fcopy,
                            scale=dcol[:, w:w + 1],
                        )
                        nc.sync.dma_start(
                            out=dest[w * P:(w + 1) * P, :], in_=o[:]
                        )

    # send1 gathered inside stage-1 (two-point interleave)
    nc.gpsimd.collective_compute(
        "AllToAll", mybir.AluOpType.bypass,
        ins=[send1[:]], outs=[table1[NPC_PAD:TROWS, :]],
        replica_groups=[list(range(N_CORES))],
    ).then_inc(cc_sem, 1)
    nc.gpsimd.wait_ge(cc_sem, 1)

    agg_layer(table1, table2, layer1=True)

    send_gather(table2, send2)
    nc.gpsimd.collective_compute(
        "AllToAll", mybir.AluOpType.bypass,
        ins=[send2[:]], outs=[table2[NPC_PAD:TROWS, :]],
        replica_groups=[list(range(N_CORES))],
    ).then_inc(cc_sem, 1)
    nc.gpsimd.wait_ge(cc_sem, 2)

    agg_layer(table2, out_pre, layer1=False)

    nc.finalize()
    return nc


def _host_epilogue(res, aux):
    pre = np.empty((N_NODES, HID_CH), dtype=np.float32)
    for c in range(N_CORES):
        o = res.results[c]["out_pre"]
        pre[c * NPC:(c + 1) * NPC] = o[aux["ranks"][c]]
    counts = aux["counts"]
    sums = np.add.reduceat(pre, aux["seg_starts"], axis=0)
    sums[counts == 0] = 0.0
    pooled = sums / np.maximum(counts, 1)[:, None]
    out = pooled.astype(np.float32) @ aux["W2"] + aux["b2"]
    out[counts == 0] = 0.0
    return out.astype(np.float32)


def kernel(**inputs):
    from concourse.bass_utils import run_bass_kernel_spmd

    meta, in_maps, aux = _prep(
        inputs["x"], inputs["W1"], inputs["b1"], inputs["W2"], inputs["b2"],
        inputs["edge_index"], inputs["batch"],
    )
    nc = _build(meta)
    res = run_bass_kernel_spmd(nc, in_maps, list(range(N_CORES)))
    return _host_epilogue(res, aux)
